# revision 18
# baseline (speedup 1.0000x reference)
"""Trainium2 Bass kernel for nn_Net_91268055040039 (dense_mlp).

Computes out[b] = sum_{t,p} x[b,t,p] * |W[t,p]| * fc1_w[0, t*P+p] + fc1_b
  x: [32, 400, 10000] f32, W: [400, 10000] f32, fc1_w: [1, 4000000] f32.

Strategy: shard the reduction dim T=400 into 8 slices of 50 rows. The kernel
is HBM-bandwidth bound (512MB of x), so x is streamed as FP16 (half the
bytes; rel err ~2e-3 vs the 2e-2 gate - inputs are N(0,1), errors average
out over the 4M-term sum). v = |W|*fc1 is precomputed on the host, fp16.

The multiply+reduce runs on the TENSOR engine (DVE scalar_tensor_tensor has
no 2x fp16 uop - measured 4.2us per 3908-elem op = 135us total, the
bottleneck of the previous version). Layout is k-major: partition p holds
k = n*128 + p, so PE contracts 128 k-values per matmul. To beat the
60-cycle-per-matmul floor, G=8 k-groups share one matmul via the diagonal
trick: lhsT = v[:, n:n+8] (8 cols), rhs = x[:, (n,b) block] [128, 8*32],
accumulating psum[8, 256] over all 489 groups; only the g==g' diagonal
[g, g*32:(g+1)*32] is wanted (the off-diag products are discarded at the
end). PE cost ~= 256 rows * 0.42ns * 489 = 53us < ~85us DMA floor.

DMA: per-partition contiguous run = chunk_n*32*2 bytes, so chunks of ~976
n-groups give 62.5KB runs (descriptor overhead dominates short runs). The
chunk schedule tapers so the compute tail past the final DMA is tiny.

End per core: 8 scalar copies extract the psum diagonal -> [8, 32] sbuf,
ones[8,1].T @ that -> [1, 32]. Host sums the 8 per-core partials + fc1_b.
"""

import numpy as np

import concourse.bass as bass
import concourse.bacc as bacc
import concourse.mybir as mybir
from concourse.tile import TileContext
from concourse.bass_utils import run_bass_kernel_spmd

B, T, P = 32, 400, 10000
NCORES = 8
TS = T // NCORES          # 50 T-rows per core
K = TS * P                # 500000 reduction elements per core per batch
PART = 128
G = 8                     # k-groups (of 128) packed per matmul
NJG = 3912                # ceil(K/128)=3907 rounded up to a multiple of G
KPAD = NJG * PART         # 500736 (736 zero pad)
# n-groups per DMA (sum = NJG). Geometric taper: chunk i small enough that
# PE's remaining work (15.6ns/group) fits inside DMA's remaining stream time
# (19ns/group) plus a ~1.5us tail - keeps the tensor engine off the critical
# path at the end. Short per-partition runs are NOT slow on this HWDGE path
# (measured 26-27 GB/s/engine down to 1KB runs).
CHUNKS = (488, 488, 488, 440, 400, 360, 296, 240, 200, 160, 144, 120, 88)
CHUNK_MAX = max(CHUNKS)
F32 = mybir.dt.float32
F16 = mybir.dt.float16

# Set by the test harness to capture an NTFF profile; harmless when False.
TRACE = False
LAST_RESULT = None


def build_program() -> bass.Bass:
    # Bacc (not raw Bass): its compile() splits multi-sem waits into separate
    # instructions - this neuronxcc build allows only 1 sync-wait per inst.
    nc = bacc.Bacc()
    xs = nc.declare_dram_parameter("xs", [PART, NJG * B], F16, isOutput=False)
    vd = nc.declare_dram_parameter("vd", [PART, NJG], F16, isOutput=False)
    out = nc.declare_dram_parameter("out", [G, G * B], F32, isOutput=True)

    with TileContext(nc) as tc:
        with (
            tc.tile_pool(name="const", bufs=1) as cpool,
            tc.tile_pool(name="xp", bufs=6) as xpool,
            tc.tile_pool(name="psum", bufs=1, space="PSUM") as ppool,
        ):
            # v first on the same sync/HWDGE ring as x (2.3us for 1MB).
            # Any cross-ring overlap (scalar HWDGE or gpsimd SWDGE) makes the
            # engines round-robin v's packets against the x stream, delaying
            # chunk completions and starving the tensor engine - measured
            # +3us (scalar) and +huge (gpsimd) vs this layout.
            vt = cpool.tile([PART, NJG], F16)
            nc.sync.dma_start(out=vt, in_=vd[:, :])

            ps = ppool.tile([G, G * B], F32)
            nmm = NJG // G
            mm = 0
            n0 = 0
            for cn in CHUNKS:
                xt = xpool.tile([PART, CHUNK_MAX * B], F16, tag="xt")
                # All x chunks on the single sync/HWDGE ring: alternating the
                # two HWDGE rings made consecutive chunks stream concurrently
                # at half rate each, delaying every completion (119us vs 97).
                nc.sync.dma_start(
                    out=xt[:, : cn * B], in_=xs[:, n0 * B : (n0 + cn) * B]
                )
                for i in range(cn // G):
                    nc.tensor.matmul(
                        out=ps,
                        lhsT=vt[:, n0 + i * G : n0 + (i + 1) * G],
                        rhs=xt[:, i * G * B : (i + 1) * G * B],
                        start=(mm == 0),
                        stop=(mm == nmm - 1),
                    )
                    mm += 1
                n0 += cn

            # Ship the whole [G, G*B] accumulator; the host extracts the
            # diagonal blocks psum[g, g*B:(g+1)*B] and sums over g (8KB out).
            res = cpool.tile([G, G * B], F32)
            nc.scalar.copy(res, ps)
            nc.sync.dma_start(out=out[:, :], in_=res)
    nc.finalize()
    return nc


def make_in_maps(x: np.ndarray, W: np.ndarray, fc1_w: np.ndarray):
    x = np.asarray(x)
    v_full = np.abs(np.asarray(W, dtype=np.float32)) * np.asarray(
        fc1_w, dtype=np.float32
    ).reshape(T, P)
    in_maps = []
    for c in range(NCORES):
        t0 = c * TS
        # x k-major: xs[p, n*B + b] = x[b, k=n*128+p]
        xpad = np.zeros((B, KPAD), dtype=np.float16)
        xpad[:, :K] = x[:, t0 : t0 + TS, :].reshape(B, K)
        xs = np.ascontiguousarray(
            xpad.reshape(B, NJG, PART).transpose(2, 1, 0)
        ).reshape(PART, NJG * B)
        vpad = np.zeros(KPAD, dtype=np.float16)
        vpad[:K] = v_full[t0 : t0 + TS].reshape(-1)
        vs = np.ascontiguousarray(vpad.reshape(NJG, PART).T)
        in_maps.append({"xs": xs, "vd": vs})
    return in_maps


def kernel(x, W, fc1_w, fc1_b):
    global LAST_RESULT
    nc = build_program()
    in_maps = make_in_maps(x, W, fc1_w)
    res = run_bass_kernel_spmd(
        nc, in_maps, core_ids=list(range(NCORES)), trace=TRACE
    )
    LAST_RESULT = res
    partial = np.zeros(B, dtype=np.float64)
    for r in res.results:
        o = r["out"].astype(np.float64)          # [G, G*B]
        for g in range(G):
            partial += o[g, g * B : (g + 1) * B]
    out = partial.astype(np.float32) + np.float32(np.asarray(fc1_b).reshape(-1)[0])
    return out.reshape(B, 1).astype(np.float32)


# revision 20
# speedup vs baseline: 1.0103x; 1.0103x over previous
"""Trainium2 Bass kernel for nn_Net_91268055040039 (dense_mlp).

Computes out[b] = sum_{t,p} x[b,t,p] * |W[t,p]| * fc1_w[0, t*P+p] + fc1_b
  x: [32, 400, 10000] f32, W: [400, 10000] f32, fc1_w: [1, 4000000] f32.

Strategy: shard the reduction dim T=400 into 8 slices of 50 rows. The kernel
is HBM-bandwidth bound (512MB of x), so x is streamed as FP16 (half the
bytes; rel err ~2e-3 vs the 2e-2 gate - inputs are N(0,1), errors average
out over the 4M-term sum). v = |W|*fc1 is precomputed on the host, fp16.

The multiply+reduce runs on the TENSOR engine (DVE scalar_tensor_tensor has
no 2x fp16 uop - measured 4.2us per 3908-elem op = 135us total, the
bottleneck of the previous version). Layout is k-major: partition p holds
k = n*128 + p, so PE contracts 128 k-values per matmul. To beat the
60-cycle-per-matmul floor, G=8 k-groups share one matmul via the diagonal
trick: lhsT = v[:, n:n+8] (8 cols), rhs = x[:, (n,b) block] [128, 8*32],
accumulating psum[8, 256] over all 489 groups; only the g==g' diagonal
[g, g*32:(g+1)*32] is wanted (the off-diag products are discarded at the
end). PE cost ~= 256 rows * 0.42ns * 489 = 53us < ~85us DMA floor.

DMA: per-partition contiguous run = chunk_n*32*2 bytes, so chunks of ~976
n-groups give 62.5KB runs (descriptor overhead dominates short runs). The
chunk schedule tapers so the compute tail past the final DMA is tiny.

End per core: 8 scalar copies extract the psum diagonal -> [8, 32] sbuf,
ones[8,1].T @ that -> [1, 32]. Host sums the 8 per-core partials + fc1_b.
"""

import numpy as np

import concourse.bass as bass
import concourse.bacc as bacc
import concourse.mybir as mybir
from concourse.tile import TileContext
from concourse.bass_utils import run_bass_kernel_spmd

B, T, P = 32, 400, 10000
NCORES = 8
TS = T // NCORES          # 50 T-rows per core
K = TS * P                # 500000 reduction elements per core per batch
PART = 128
G = 8                     # k-groups (of 128) packed per matmul
NJG = 3912                # ceil(K/128)=3907 rounded up to a multiple of G
KPAD = NJG * PART         # 500736 (736 zero pad)
# n-groups per DMA (sum = NJG). Geometric taper: chunk i small enough that
# PE's remaining work (15.6ns/group) fits inside DMA's remaining stream time
# (19ns/group) plus a ~1.5us tail - keeps the tensor engine off the critical
# path at the end. Short per-partition runs are NOT slow on this HWDGE path
# (measured 26-27 GB/s/engine down to 1KB runs).
CHUNKS = (488, 488, 488, 440, 400, 360, 296, 240, 200, 160, 136, 104, 64, 48)
CHUNK_MAX = max(CHUNKS)
F32 = mybir.dt.float32
F16 = mybir.dt.float16

# Set by the test harness to capture an NTFF profile; harmless when False.
TRACE = False
LAST_RESULT = None


def build_program() -> bass.Bass:
    # Bacc (not raw Bass): its compile() splits multi-sem waits into separate
    # instructions - this neuronxcc build allows only 1 sync-wait per inst.
    nc = bacc.Bacc()
    xs = nc.declare_dram_parameter("xs", [PART, NJG * B], F16, isOutput=False)
    vd = nc.declare_dram_parameter("vd", [PART, NJG], F16, isOutput=False)
    out = nc.declare_dram_parameter("out", [G, G * B], F32, isOutput=True)

    with TileContext(nc) as tc:
        with (
            tc.tile_pool(name="const", bufs=1) as cpool,
            tc.tile_pool(name="xp", bufs=6) as xpool,
            tc.tile_pool(name="psum", bufs=1, space="PSUM") as ppool,
        ):
            # v first on the same sync/HWDGE ring as x (2.3us for 1MB).
            # Any cross-ring overlap (scalar HWDGE or gpsimd SWDGE) makes the
            # engines round-robin v's packets against the x stream, delaying
            # chunk completions and starving the tensor engine - measured
            # +3us (scalar) and +huge (gpsimd) vs this layout.
            vt = cpool.tile([PART, NJG], F16)
            nc.sync.dma_start(out=vt, in_=vd[:, :])

            ps = ppool.tile([G, G * B], F32)
            nmm = NJG // G
            mm = 0
            n0 = 0
            for ci, cn in enumerate(CHUNKS):
                xt = xpool.tile([PART, CHUNK_MAX * B], F16, tag="xt")
                # Chunk 0 on the gpsimd/SWDGE ring: its Q7 dispatch starts
                # ~1.5us before the sync/HWDGE ring's first byte, pulling the
                # whole stream earlier (engines merge the two queues). All
                # later chunks on the single sync ring: alternating the two
                # HWDGE rings made consecutive chunks stream concurrently at
                # half rate each, delaying every completion (119us vs 97).
                eng = nc.gpsimd if ci == 0 else nc.sync
                eng.dma_start(
                    out=xt[:, : cn * B], in_=xs[:, n0 * B : (n0 + cn) * B]
                )
                for i in range(cn // G):
                    nc.tensor.matmul(
                        out=ps,
                        lhsT=vt[:, n0 + i * G : n0 + (i + 1) * G],
                        rhs=xt[:, i * G * B : (i + 1) * G * B],
                        start=(mm == 0),
                        stop=(mm == nmm - 1),
                    )
                    mm += 1
                n0 += cn

            # Ship the whole [G, G*B] accumulator; the host extracts the
            # diagonal blocks psum[g, g*B:(g+1)*B] and sums over g (8KB out).
            res = cpool.tile([G, G * B], F32)
            nc.scalar.copy(res, ps)
            nc.sync.dma_start(out=out[:, :], in_=res)
    nc.finalize()
    return nc


def make_in_maps(x: np.ndarray, W: np.ndarray, fc1_w: np.ndarray):
    x = np.asarray(x)
    v_full = np.abs(np.asarray(W, dtype=np.float32)) * np.asarray(
        fc1_w, dtype=np.float32
    ).reshape(T, P)
    in_maps = []
    for c in range(NCORES):
        t0 = c * TS
        # x k-major: xs[p, n*B + b] = x[b, k=n*128+p]
        xpad = np.zeros((B, KPAD), dtype=np.float16)
        xpad[:, :K] = x[:, t0 : t0 + TS, :].reshape(B, K)
        xs = np.ascontiguousarray(
            xpad.reshape(B, NJG, PART).transpose(2, 1, 0)
        ).reshape(PART, NJG * B)
        vpad = np.zeros(KPAD, dtype=np.float16)
        vpad[:K] = v_full[t0 : t0 + TS].reshape(-1)
        vs = np.ascontiguousarray(vpad.reshape(NJG, PART).T)
        in_maps.append({"xs": xs, "vd": vs})
    return in_maps


def kernel(x, W, fc1_w, fc1_b):
    global LAST_RESULT
    nc = build_program()
    in_maps = make_in_maps(x, W, fc1_w)
    res = run_bass_kernel_spmd(
        nc, in_maps, core_ids=list(range(NCORES)), trace=TRACE
    )
    LAST_RESULT = res
    partial = np.zeros(B, dtype=np.float64)
    for r in res.results:
        o = r["out"].astype(np.float64)          # [G, G*B]
        for g in range(G):
            partial += o[g, g * B : (g + 1) * B]
    out = partial.astype(np.float32) + np.float32(np.asarray(fc1_b).reshape(-1)[0])
    return out.reshape(B, 1).astype(np.float32)
